# revision 5
# baseline (speedup 1.0000x reference)
"""2-layer LSTM (B=128, T=1024, H=256) + last-step LayerNorm on 8 trn2 cores.

Data-parallel over batch (16 rows/core). Per core, everything is kept in a
transposed layout (hidden/gate dims on partitions, batch on the free axis):

  - gates.T for a block of 8 timesteps live in one PSUM tile [128, 1024]
    (col = 128*chunk + 16*step_in_block + b). Per block, the bias is first
    seeded with a single [8,128]-stationary x one-hot matmul (1 ldweights
    instead of 8), then the input projection x @ Wih.T is matmul-accumulated
    (Wih.T chunks stationary, x.T streaming). Each timestep's recurrent term
    then accumulates into its 16-column slice (Whh.T chunks stationary, h.T
    streaming), so no separate add is ever needed.
  - all four gates use ONE sigmoid instruction per step: tanh(x) is computed
    as 2*sigmoid(2x)-1 by pre-doubling the g-gate rows of the weights on the
    host, and h is stored as h/2 = (sig(2c)-0.5)*o with the missing factor 2
    folded into the next consumer's weight columns (Whh, layer-2 Wih); the
    final LayerNorm is scale-invariant so h/2 needs no correction there.
  - cell update per step: DVE ig2=(sg-.5)*i, Pool fc=f*c, DVE c=(2*ig2)+fc,
    ACT sc=sig(2c), DVE h=(sc-.5)*o written in bf16 directly where the next
    matmul streams it from.
  - layer 2 runs 8 steps behind layer 1 so its per-8-step x-projection
    (from layer 1's h history ring) is ready, and so PE/ACT/DVE work of
    the two layers overlaps.

Matmuls are bf16 (fp32 PSUM accumulate); c stays fp32. Final step: PE
transpose of h2 back to [16, 256], LayerNorm, DMA out.
"""
import sys

sys.path.insert(0, "/opt/trn_rl_repo")

import numpy as np
import ml_dtypes

import concourse.bass as bass
import concourse.mybir as mybir
import concourse.tile as tile
from concourse.alu_op_type import AluOpType

NUM_LAYERS = 2
H = 256
F = 256
B, T = 128, 1024
LN_EPS = 1e-5
N_CORES = 8
BL = B // N_CORES          # batch rows per core = 16
G4 = 4 * H                 # 1024 gate dims
NCH = G4 // 128            # 8 chunks of gate dims
SBLK = 8                   # timesteps per x-projection block
BF16 = mybir.dt.bfloat16
F32 = mybir.dt.float32

# gate reorder: torch (i,f,g,o) -> (i,f,o,g) so the sig-tile slices are
# i=[0:2H], f=[2H? no: cols] ... chunks (i0,i1,f0,f1,o0,o1,g0,g1)
PERM = np.concatenate([np.arange(0, 2 * H), np.arange(3 * H, 4 * H),
                       np.arange(2 * H, 3 * H)])


def _split_excess_waits(nc):
    """walrus in this container rejects instructions with >2 sem waits
    (CoreV3 setupSyncWait). Hoist excess waits onto NoOps just before."""
    for fn in nc.m.functions:
        for blk in fn.blocks:
            insts = list(blk.instructions)
            out, n_new = [], 0
            for inst in insts:
                si = inst.sync_info
                waits = list(si.on_wait) if si is not None else []
                if len(waits) > 1:
                    head, rest = waits[:-1], waits[-1:]
                    # chain NoOps, one wait each (1-wait-per-inst walrus limit)
                    for wt in head:
                        nop = mybir.InstNoOp(
                            name=f"{inst.name}-ws{n_new}",
                            engine=inst.engine,
                            ins=[], outs=[],
                            sync_info=mybir.SyncInfo(on_wait=[wt], on_update=[]),
                        )
                        n_new += 1
                        out.append(nop)
                    inst.sync_info = mybir.SyncInfo(
                        on_wait=rest, on_update=list(si.on_update))
                out.append(inst)
            if n_new:
                try:
                    blk.instructions = out
                except Exception:
                    blk.set_instructions(out)


def build(t_steps=T, split_waits=True):
    nc = bass.Bass()
    TB = t_steps * BL
    xt_d = nc.dram_tensor("xt", [2, 128, TB], BF16, kind="ExternalInput")
    wih_d = nc.dram_tensor("wih", [NUM_LAYERS, 2, 128, G4], BF16, kind="ExternalInput")
    whh_d = nc.dram_tensor("whh", [NUM_LAYERS, 2, 128, G4], BF16, kind="ExternalInput")
    b8_d = nc.dram_tensor("b8", [NUM_LAYERS, NCH, 128], BF16, kind="ExternalInput")
    sel_d = nc.dram_tensor("sel", [NCH, G4], BF16, kind="ExternalInput")
    ident_d = nc.dram_tensor("ident", [128, 128], F32, kind="ExternalInput")
    gam_d = nc.dram_tensor("gam", [BL, H], F32, kind="ExternalInput")
    bet_d = nc.dram_tensor("bet", [BL, H], F32, kind="ExternalInput")
    y_d = nc.dram_tensor("y", [BL, H], F32, kind="ExternalOutput")

    NB = t_steps // SBLK
    with tile.TileContext(nc) as tc:
        with (
            tc.tile_pool(name="wts", bufs=1) as wts,
            tc.tile_pool(name="state", bufs=1) as st,
            tc.tile_pool(name="work", bufs=4) as wk,
            tc.tile_pool(name="psum", bufs=2, space="PSUM") as ps,
        ):
            # resident tensors (partition dim first on every SBUF tile)
            xt = [wts.tile([128, TB], BF16, tag=f"xt{kw}", name=f"xt{kw}") for kw in (0, 1)]
            for kw in (0, 1):
                nc.sync.dma_start(xt[kw][:], xt_d[kw])
            wih = [[wts.tile([128, G4], BF16, tag=f"wih{l}{kw}", name=f"wih{l}{kw}") for kw in (0, 1)]
                   for l in range(NUM_LAYERS)]
            whh = [[wts.tile([128, G4], BF16, tag=f"whh{l}{kw}", name=f"whh{l}{kw}") for kw in (0, 1)]
                   for l in range(NUM_LAYERS)]
            b8 = [wts.tile([NCH, 128], BF16, tag=f"b8{l}", name=f"b8{l}") for l in range(NUM_LAYERS)]
            sel = wts.tile([NCH, G4], BF16, tag="sel", name="sel")
            nc.sync.dma_start(sel[:], sel_d[:])
            for l in range(NUM_LAYERS):
                for kw in (0, 1):
                    nc.sync.dma_start(wih[l][kw][:], wih_d[l, kw])
                    nc.sync.dma_start(whh[l][kw][:], whh_d[l, kw])
                nc.sync.dma_start(b8[l][:], b8_d[l])
            ident = wts.tile([128, 128], F32, tag="ident", name="ident")
            nc.sync.dma_start(ident[:], ident_d[:])
            zb = wts.tile([128, 1], F32, tag="zb", name="zb")
            nc.vector.memset(zb[:], 0.0)
            eps16 = wts.tile([16, 1], F32, tag="eps16", name="eps16")
            nc.vector.memset(eps16[:], LN_EPS)

            # persistent state
            hist = st.tile([128, SBLK * 32], BF16, tag="hist", name="hist")   # layer-1 h ring
            h2 = st.tile([128, 32], BF16, tag="h2", name="h2")
            c1 = st.tile([128, 32], F32, tag="c1", name="c1")
            c2 = st.tile([128, 32], F32, tag="c2", name="c2")
            h2f = st.tile([128, 32], F32, tag="h2f", name="h2f")

            xp_cur = [None, None]   # current psum block tile per layer

            def xproj_block(l, k):
                """bias + Wih_l @ x_l.T for steps [8k, 8k+8) into psum."""
                xp = ps.tile([128, SBLK * 128], F32, tag=f"xp{l}", name=f"xp{l}")
                xp_cur[l] = xp
                # seed with bias: one [8,128] stationary, one-hot stream
                # (PSUM bank limit: 512 f32 cols per matmul)
                for hb in (0, 1):
                    nc.tensor.matmul(
                        xp[:, hb * 512:(hb + 1) * 512], b8[l][:],
                        sel[:, hb * 512:(hb + 1) * 512],
                        start=True, stop=False, skip_group_check=True)
                if l == 0:
                    rhs = [xt[kw][:, k * SBLK * BL:(k + 1) * SBLK * BL] for kw in (0, 1)]
                else:
                    hv = hist[:].rearrange("p (s w) -> p s w", s=SBLK)
                    rhs = [hv[:, :, 0:BL], hv[:, :, BL:2 * BL]]
                for ch in range(NCH):
                    o = xp[:, ch * 128:(ch + 1) * 128]
                    for kw in (0, 1):
                        nc.tensor.matmul(
                            o, wih[l][kw][:, ch * 128:(ch + 1) * 128], rhs[kw],
                            start=False, stop=(kw == 1 and ch == NCH - 1),
                            skip_group_check=True)

            def step(l, t):
                """one LSTM timestep in transposed layout."""
                k, s = t // SBLK, t % SBLK
                if s == 0:
                    xproj_block(l, k)
                xp = xp_cur[l]
                c_t = c1 if l == 0 else c2
                if t > 0:
                    if l == 0:
                        hsrc = hist[:, ((t - 1) % SBLK) * 32:((t - 1) % SBLK) * 32 + 32]
                    else:
                        hsrc = h2[:]
                    for ch in range(NCH):
                        o = xp[:, ch * 128 + 16 * s: ch * 128 + 16 * s + 16]
                        for kw in (0, 1):
                            nc.tensor.matmul(
                                o, whh[l][kw][:, ch * 128:(ch + 1) * 128],
                                hsrc[:, 16 * kw:16 * kw + 16],
                                start=False, stop=(kw == 1), skip_group_check=True)
                xpv = xp[:].rearrange("p (c s w) -> p c s w", c=NCH, s=SBLK)
                # ONE sigmoid for all gates: cols (i0,i1,f0,f1,o0,o1,sg0,sg1)
                sig = wk.tile([128, 128], F32, tag="sig", name="sig")
                nc.scalar.activation(sig[:].rearrange("p (c w) -> p c w", c=NCH),
                                     xpv[:, 0:NCH, s, :],
                                     mybir.ActivationFunctionType.Sigmoid,
                                     bias=zb[:])
                # ig2 = (sg - 0.5) * i   (= i*g/2)
                ig2 = wk.tile([128, 32], F32, tag="ig2", name="ig2")
                nc.vector.scalar_tensor_tensor(
                    ig2[:], sig[:, 96:128], 0.5, sig[:, 0:32],
                    AluOpType.subtract, AluOpType.mult)
                if t > 0:
                    # fc = f * c (Pool engine, parallel with ig2 on DVE)
                    fc = wk.tile([128, 32], F32, tag="fc", name="fc")
                    nc.gpsimd.tensor_mul(fc[:], sig[:, 32:64], c_t[:])
                    # c = 2*ig2 + fc
                    nc.vector.scalar_tensor_tensor(
                        c_t[:], ig2[:], 2.0, fc[:],
                        AluOpType.mult, AluOpType.add)
                else:
                    nc.vector.tensor_scalar_mul(c_t[:], ig2[:], 2.0)
                # tc = tanh(c); h/2 = (tc * 0.5) * o
                sc = wk.tile([128, 32], F32, tag="sc", name="sc")
                nc.scalar.activation(sc[:], c_t[:],
                                     mybir.ActivationFunctionType.Tanh,
                                     bias=zb[:])
                if l == 0:
                    hdst = hist[:, (t % SBLK) * 32:(t % SBLK) * 32 + 32]
                else:
                    hdst = h2[:]
                nc.vector.scalar_tensor_tensor(
                    hdst, sc[:], 0.5, sig[:, 64:96],
                    AluOpType.mult, AluOpType.mult)
                if l == 1 and t == t_steps - 1:
                    nc.vector.scalar_tensor_tensor(
                        h2f[:], sc[:], 0.5, sig[:, 64:96],
                        AluOpType.mult, AluOpType.mult)

            for w in range(t_steps + SBLK):
                if SBLK <= w:
                    step(1, w - SBLK)
                if w < t_steps:
                    step(0, w)

            # ---- LayerNorm over H on h2f (h2.T layout, h/2 scale) -> y [16, 256]
            pt = ps.tile([16, 256], F32, tag="xp0", name="xp0")
            nc.tensor.transpose(pt[:, 0:128], h2f[:, 0:16], ident[:])
            nc.tensor.transpose(pt[:, 128:256], h2f[:, 16:32], ident[:])
            hb_ = wk.tile([16, 256], F32, tag="hb", name="hb")
            nc.vector.tensor_copy(hb_[:], pt[:])
            dum = wk.tile([16, 256], F32, tag="dum", name="dum")
            acc = wk.tile([16, 1], F32, tag="acc", name="acc")
            nc.scalar.activation(dum[:], hb_[:], mybir.ActivationFunctionType.Copy,
                                 accum_out=acc[:])
            mu = wk.tile([16, 1], F32, tag="mu", name="mu")
            nc.vector.tensor_scalar_mul(mu[:], acc[:], 1.0 / H)
            cen = wk.tile([16, 256], F32, tag="cen", name="cen")
            nc.vector.tensor_scalar_sub(cen[:], hb_[:], mu[:])
            acc2 = wk.tile([16, 1], F32, tag="acc2", name="acc2")
            nc.scalar.activation(dum[:], cen[:], mybir.ActivationFunctionType.Square,
                                 bias=zb[0:16, :], accum_out=acc2[:])
            sd = wk.tile([16, 1], F32, tag="sd", name="sd")
            nc.scalar.activation(sd[:], acc2[:], mybir.ActivationFunctionType.Sqrt,
                                 scale=1.0 / H, bias=eps16[:])
            rstd = wk.tile([16, 1], F32, tag="rstd", name="rstd")
            nc.vector.reciprocal(rstd[:], sd[:])
            nrm = wk.tile([16, 256], F32, tag="nrm", name="nrm")
            nc.vector.tensor_scalar_mul(nrm[:], cen[:], rstd[:])
            gam = wk.tile([16, 256], F32, tag="gam", name="gam")
            nc.sync.dma_start(gam[:], gam_d[:])
            bet = wk.tile([16, 256], F32, tag="bet", name="bet")
            nc.sync.dma_start(bet[:], bet_d[:])
            nc.vector.tensor_mul(nrm[:], nrm[:], gam[:])
            out = wk.tile([16, 256], F32, tag="out", name="out")
            nc.vector.tensor_add(out[:], nrm[:], bet[:])
            nc.sync.dma_start(y_d[:], out[:])

    if split_waits:
        _split_excess_waits(nc)
    return nc


def prep_inputs(x, W_ih, W_hh, b_ih, b_hh, ln_gamma, ln_beta, t_steps=T):
    """host-side shard + transpose + cast + rescale. Per-core input dicts.

    Rescaling for the all-sigmoid cell (tanh(v) = 2*sig(2v)-1, h stored
    as h/2):
      - g-gate rows of W_ih, W_hh, bias doubled (sig sees 2*xg)
      - Whh columns doubled (consumes h/2), layer-2 Wih columns doubled
    """
    bf = ml_dtypes.bfloat16
    Wih = np.asarray(W_ih, dtype=np.float64)[:, PERM, :].copy()
    Whh = np.asarray(W_hh, dtype=np.float64)[:, PERM, :].copy()
    bias = (np.asarray(b_ih, dtype=np.float64) + np.asarray(b_hh, dtype=np.float64))[:, PERM].copy()
    # g rows doubled (g block = rows 3H:4H after PERM)
    Wih[:, 3 * H:, :] *= 2.0
    Whh[:, 3 * H:, :] *= 2.0
    bias[:, 3 * H:] *= 2.0
    # h/2 compensation on consumer columns
    Whh *= 2.0
    Wih[1] *= 2.0

    wih = np.ascontiguousarray(np.transpose(Wih, (0, 2, 1))).reshape(NUM_LAYERS, 2, 128, G4)
    whh = np.ascontiguousarray(np.transpose(Whh, (0, 2, 1))).reshape(NUM_LAYERS, 2, 128, G4)
    b8 = bias.reshape(NUM_LAYERS, NCH, 128)
    selm = np.zeros((NCH, G4), dtype=np.float32)
    for kk in range(NCH):
        selm[kk, kk * 128:(kk + 1) * 128] = 1.0
    ident = np.eye(128, dtype=np.float32)
    ins = []
    for cid in range(N_CORES):
        xs = x[cid * BL:(cid + 1) * BL, :t_steps, :]        # [16, t, 256]
        xtp = np.transpose(xs, (2, 1, 0)).reshape(F, t_steps * BL)  # [256, t*16]
        ins.append({
            "xt": np.ascontiguousarray(xtp.reshape(2, 128, t_steps * BL)).astype(bf),
            "wih": wih.astype(bf), "whh": whh.astype(bf),
            "b8": b8.astype(bf), "sel": selm.astype(bf), "ident": ident,
            "gam": np.broadcast_to(ln_gamma, (BL, H)).astype(np.float32).copy(),
            "bet": np.broadcast_to(ln_beta, (BL, H)).astype(np.float32).copy(),
        })
    return ins


_CACHED = {}


def kernel(x, W_ih, W_hh, b_ih, b_hh, ln_gamma, ln_beta):
    from concourse.bass_utils import run_bass_kernel_spmd
    x = np.asarray(x, dtype=np.float32)
    ins = prep_inputs(np.asarray(x), np.asarray(W_ih), np.asarray(W_hh),
                      np.asarray(b_ih), np.asarray(b_hh),
                      np.asarray(ln_gamma), np.asarray(ln_beta))
    if "nc" not in _CACHED:
        _CACHED["nc"] = build(T)
    res = run_bass_kernel_spmd(_CACHED["nc"], ins, core_ids=list(range(N_CORES)))
    return np.concatenate([res.results[c]["y"] for c in range(N_CORES)], axis=0)


# revision 18
# speedup vs baseline: 1.4821x; 1.4821x over previous
"""2-layer LSTM (B=128, T=1024, H=256) + last-step LayerNorm on 8 trn2 cores.

Data-parallel over batch (16 rows/core). Per core, everything is kept in a
transposed layout (hidden/gate dims on partitions, batch on the free axis):

  - gates.T for a block of 8 timesteps live in one PSUM tile [128, 1024]
    (col = 128*chunk + 16*step_in_block + b). Per block, the bias is first
    seeded with a single [8,128]-stationary x one-hot matmul (1 ldweights
    instead of 8), then the input projection x @ Wih.T is matmul-accumulated
    (Wih.T chunks stationary, x.T streaming). Each timestep's recurrent term
    then accumulates into its 16-column slice (Whh.T chunks stationary, h.T
    streaming), so no separate add is ever needed.
  - all four gates use ONE sigmoid instruction per step: tanh(x) is computed
    as 2*sigmoid(2x)-1 by pre-doubling the g-gate rows of the weights on the
    host, and h is stored as h/2 = (sig(2c)-0.5)*o with the missing factor 2
    folded into the next consumer's weight columns (Whh, layer-2 Wih); the
    final LayerNorm is scale-invariant so h/2 needs no correction there.
  - cell update per step: DVE ig2=(sg-.5)*i, Pool fc=f*c, DVE c=(2*ig2)+fc,
    ACT sc=sig(2c), DVE h=(sc-.5)*o written in bf16 directly where the next
    matmul streams it from.
  - layer 2 runs 8 steps behind layer 1 so its per-8-step x-projection
    (from layer 1's h history ring) is ready, and so PE/ACT/DVE work of
    the two layers overlaps.

Matmuls are bf16 (fp32 PSUM accumulate); c stays fp32. Final step: PE
transpose of h2 back to [16, 256], LayerNorm, DMA out.
"""
import sys

sys.path.insert(0, "/opt/trn_rl_repo")

import numpy as np
import ml_dtypes

import concourse.bass as bass
import concourse.mybir as mybir
import concourse.tile as tile
from concourse.alu_op_type import AluOpType

NUM_LAYERS = 2
H = 256
F = 256
B, T = 128, 1024
LN_EPS = 1e-5
N_CORES = 8
BL = B // N_CORES          # batch rows per core = 16
G4 = 4 * H                 # 1024 gate dims
NCH = G4 // 128            # 8 chunks of gate dims
SBLK = 8                   # timesteps per x-projection block
BF16 = mybir.dt.bfloat16
F32 = mybir.dt.float32

# gate reorder: torch (i,f,g,o) -> (i,f,o,g) so the sig-tile slices are
# i=[0:2H], f=[2H? no: cols] ... chunks (i0,i1,f0,f1,o0,o1,g0,g1)
PERM = np.concatenate([np.arange(0, 2 * H), np.arange(3 * H, 4 * H),
                       np.arange(2 * H, 3 * H)])


def _split_excess_waits(nc):
    """walrus in this container rejects instructions with >2 sem waits
    (CoreV3 setupSyncWait). Hoist excess waits onto NoOps just before."""
    for fn in nc.m.functions:
        for blk in fn.blocks:
            insts = list(blk.instructions)
            out, n_new = [], 0
            for inst in insts:
                si = inst.sync_info
                waits = list(si.on_wait) if si is not None else []
                if len(waits) > 1:
                    head, rest = waits[:-1], waits[-1:]
                    # chain NoOps, one wait each (1-wait-per-inst walrus limit)
                    for wt in head:
                        nop = mybir.InstNoOp(
                            name=f"{inst.name}-ws{n_new}",
                            engine=inst.engine,
                            ins=[], outs=[],
                            sync_info=mybir.SyncInfo(on_wait=[wt], on_update=[]),
                        )
                        n_new += 1
                        out.append(nop)
                    inst.sync_info = mybir.SyncInfo(
                        on_wait=rest, on_update=list(si.on_update))
                out.append(inst)
            if n_new:
                try:
                    blk.instructions = out
                except Exception:
                    blk.set_instructions(out)


def build(t_steps=T, split_waits=True):
    nc = bass.Bass()
    TB = t_steps * BL
    xt_d = nc.dram_tensor("xt", [2, 128, TB], BF16, kind="ExternalInput")
    wih_d = nc.dram_tensor("wih", [NUM_LAYERS, 2, 128, G4], BF16, kind="ExternalInput")
    whh_d = nc.dram_tensor("whh", [NUM_LAYERS, 2, 128, G4], BF16, kind="ExternalInput")
    bbrd_d = nc.dram_tensor("bbrd", [NUM_LAYERS, 128, G4], BF16, kind="ExternalInput")
    identb_d = nc.dram_tensor("identb", [128, 128], BF16, kind="ExternalInput")
    ident_d = nc.dram_tensor("ident", [128, 128], F32, kind="ExternalInput")
    gam_d = nc.dram_tensor("gam", [BL, H], F32, kind="ExternalInput")
    bet_d = nc.dram_tensor("bet", [BL, H], F32, kind="ExternalInput")
    y_d = nc.dram_tensor("y", [BL, H], F32, kind="ExternalOutput")

    NB = t_steps // SBLK
    LAG = 12                   # layer-2 wave offset (staggers block xprojs)
    NSLOT = 2 * SBLK           # double-buffered layer-1 h history ring
    with tile.TileContext(nc) as tc:
        with (
            tc.tile_pool(name="wts", bufs=1) as wts,
            tc.tile_pool(name="state", bufs=1) as st,
            tc.tile_pool(name="work", bufs=4) as wk,
            tc.tile_pool(name="psum", bufs=2, space="PSUM") as ps,
        ):
            # resident tensors (partition dim first on every SBUF tile)
            xt = [wts.tile([128, TB], BF16, tag=f"xt{kw}", name=f"xt{kw}") for kw in (0, 1)]
            for kw in (0, 1):
                nc.sync.dma_start(xt[kw][:], xt_d[kw])
            wih = [[wts.tile([128, G4], BF16, tag=f"wih{l}{kw}", name=f"wih{l}{kw}") for kw in (0, 1)]
                   for l in range(NUM_LAYERS)]
            whh = [[wts.tile([128, G4], BF16, tag=f"whh{l}{kw}", name=f"whh{l}{kw}") for kw in (0, 1)]
                   for l in range(NUM_LAYERS)]
            bbrd = [wts.tile([128, G4], BF16, tag=f"bbrd{l}", name=f"bbrd{l}") for l in range(NUM_LAYERS)]
            identb = wts.tile([128, 128], BF16, tag="identb", name="identb")
            nc.sync.dma_start(identb[:], identb_d[:])
            for l in range(NUM_LAYERS):
                for kw in (0, 1):
                    nc.sync.dma_start(wih[l][kw][:], wih_d[l, kw])
                    nc.sync.dma_start(whh[l][kw][:], whh_d[l, kw])
                nc.sync.dma_start(bbrd[l][:], bbrd_d[l])
            ident = wts.tile([128, 128], F32, tag="ident", name="ident")
            nc.sync.dma_start(ident[:], ident_d[:])
            zb = wts.tile([128, 1], F32, tag="zb", name="zb")
            nc.vector.memset(zb[:], 0.0)
            # LN input is h/2, so var is scaled by 1/4 — scale eps to match
            eps16 = wts.tile([16, 1], F32, tag="eps16", name="eps16")
            nc.vector.memset(eps16[:], LN_EPS / 4)

            # persistent state
            hist = st.tile([128, NSLOT * 32], BF16, tag="hist", name="hist")  # layer-1 h ring
            h2 = st.tile([128, 32], BF16, tag="h2", name="h2")
            c1 = st.tile([128, 32], F32, tag="c1", name="c1")
            c2 = st.tile([128, 32], F32, tag="c2", name="c2")
            h2f = st.tile([128, 32], F32, tag="h2f", name="h2f")

            xp_cur = [None, None]    # psum tile being consumed, per layer
            xp_next = [None, None]   # psum tile being produced, per layer

            def xproj_rhs(l, k):
                if l == 0:
                    return [xt[kw][:, k * SBLK * BL:(k + 1) * SBLK * BL] for kw in (0, 1)]
                hv = hist[:].rearrange("p (s w) -> p s w", s=NSLOT)
                s0 = (k % 2) * SBLK
                return [hv[:, s0:s0 + SBLK, 0:BL], hv[:, s0:s0 + SBLK, BL:2 * BL]]

            def xproj_chunk(l, k, ch):
                """emit bias-seed (ch 0/4) + Wih matmuls for one gate chunk
                of block k into xp_next[l]."""
                if ch == 0:
                    xp_next[l] = ps.tile([128, SBLK * 128], F32, tag=f"xp{l}", name=f"xp{l}")
                xp = xp_next[l]
                if ch % 4 == 0:
                    hb = ch // 4
                    nc.tensor.matmul(
                        xp[:, hb * 512:(hb + 1) * 512], identb[:],
                        bbrd[l][:, hb * 512:(hb + 1) * 512],
                        start=True, stop=False, skip_group_check=True)
                rhs = xproj_rhs(l, k)
                o = xp[:, ch * 128:(ch + 1) * 128]
                for kw in (0, 1):
                    nc.tensor.matmul(
                        o, wih[l][kw][:, ch * 128:(ch + 1) * 128], rhs[kw],
                        start=False, stop=(kw == 1 and ch == NCH - 1),
                        skip_group_check=True)

            def rec_matmuls(l, t):
                """kw-major: all kw0 chunks first so they can start as soon
                as the h half-0 write lands."""
                xp = xp_cur[l]
                s = t % SBLK
                if l == 0:
                    hsrc = hist[:, ((t - 1) % NSLOT) * 32:((t - 1) % NSLOT) * 32 + 32]
                else:
                    hsrc = h2[:]
                for kw in (0, 1):
                    for ch in range(NCH):
                        o = xp[:, ch * 128 + 16 * s: ch * 128 + 16 * s + 16]
                        nc.tensor.matmul(
                            o, whh[l][kw][:, ch * 128:(ch + 1) * 128],
                            hsrc[:, 16 * kw:16 * kw + 16],
                            start=False, stop=(kw == 1), skip_group_check=True)

            def emit_sig(l, t):
                s = t % SBLK
                xpv = xp_cur[l][:].rearrange("p (c s w) -> p c s w", c=NCH, s=SBLK)
                sig = wk.tile([128, 128], F32, tag=f"sig{l}", name=f"sig{l}")
                nc.scalar.activation(sig[:].rearrange("p (c w) -> p c w", c=NCH),
                                     xpv[:, 0:NCH, s, :],
                                     mybir.ActivationFunctionType.Sigmoid,
                                     bias=zb[:])
                return sig

            # prologue: layer-1 block 0 (layer-2 block 0 is emitted at
            # waves 8..11 once the hist ring has filled)
            for ch in range(NCH):
                xproj_chunk(0, 0, ch)

            for w in range(t_steps + LAG):
                act = []
                if LAG <= w:
                    act.append((1, w - LAG))
                if w < t_steps:
                    act.append((0, w))

                # consume-switch psum blocks at each layer's block start
                for (l, t) in act:
                    if t % SBLK == 0:
                        xp_cur[l] = xp_next[l]

                # stage B: recurrent matmuls (l2 first: its inputs are older)
                for (l, t) in act:
                    if t > 0:
                        rec_matmuls(l, t)

                # stage A: xproj slices for upcoming blocks (PE, after recs).
                # Emitted in the LATE half of each block's 8 waves so the
                # psum-buffer WAR (bufs=2 rotation) has drained.
                if w % SBLK >= 4 and w < t_steps:
                    k1 = w // SBLK + 1
                    if k1 < NB:
                        for ch in (2 * (w % SBLK - 4), 2 * (w % SBLK - 4) + 1):
                            xproj_chunk(0, k1, ch)
                if w >= SBLK and w % SBLK <= 3:
                    k2 = (w - SBLK) // SBLK
                    if k2 < NB:
                        for ch in (2 * (w % SBLK), 2 * (w % SBLK) + 1):
                            xproj_chunk(1, k2, ch)

                # stage C: sigmoids
                sigs = {l: emit_sig(l, t) for (l, t) in act}

                # stages D/E: all-DVE cell chain per layer (back-to-back on
                # one engine avoids two cross-engine sem hops via Pool):
                #   ig2 = (sg-0.5)*i ; fc = f*c ; c = 2*ig2 + fc
                for (l, t) in act:
                    sig = sigs[l]
                    c_t = c1 if l == 0 else c2
                    ig2 = wk.tile([128, 32], F32, tag=f"ig2{l}", name=f"ig2{l}")
                    nc.vector.scalar_tensor_tensor(
                        ig2[:], sig[:, 96:128], 0.5, sig[:, 0:32],
                        AluOpType.subtract, AluOpType.mult)
                    if t > 0:
                        fc = wk.tile([128, 32], F32, tag=f"fc{l}", name=f"fc{l}")
                        nc.vector.tensor_mul(fc[:], sig[:, 32:64], c_t[:])
                        nc.vector.scalar_tensor_tensor(
                            c_t[:], ig2[:], 2.0, fc[:],
                            AluOpType.mult, AluOpType.add)
                    else:
                        nc.vector.tensor_scalar_mul(c_t[:], ig2[:], 2.0)

                # stage F: tc = tanh(c) (ACT)
                tcs = {}
                for (l, t) in act:
                    c_t = c1 if l == 0 else c2
                    sc = wk.tile([128, 32], F32, tag=f"sc{l}", name=f"sc{l}")
                    nc.scalar.activation(sc[:], c_t[:],
                                         mybir.ActivationFunctionType.Tanh,
                                         bias=zb[:])
                    tcs[l] = sc

                # stage G: h/2 = (tc*0.5)*o (DVE), bf16 into next consumer.
                # Written in two 16-col halves so the next step's kw0 rec
                # matmuls can launch while the kw1 half is still in flight.
                for (l, t) in act:
                    if l == 0:
                        hdst = hist[:, (t % NSLOT) * 32:(t % NSLOT) * 32 + 32]
                    else:
                        hdst = h2[:]
                    for kw in (0, 1):
                        nc.vector.scalar_tensor_tensor(
                            hdst[:, 16 * kw:16 * kw + 16],
                            tcs[l][:, 16 * kw:16 * kw + 16], 0.5,
                            sigs[l][:, 64 + 16 * kw:80 + 16 * kw],
                            AluOpType.mult, AluOpType.mult)
                    if l == 1 and t == t_steps - 1:
                        nc.vector.scalar_tensor_tensor(
                            h2f[:], tcs[l][:], 0.5, sigs[l][:, 64:96],
                            AluOpType.mult, AluOpType.mult)

            # ---- LayerNorm over H on h2f (h2.T layout, h/2 scale) -> y [16, 256]
            pt = ps.tile([16, 256], F32, tag="xp0", name="xp0")
            nc.tensor.transpose(pt[:, 0:128], h2f[:, 0:16], ident[:])
            nc.tensor.transpose(pt[:, 128:256], h2f[:, 16:32], ident[:])
            hb_ = wk.tile([16, 256], F32, tag="hb", name="hb")
            nc.vector.tensor_copy(hb_[:], pt[:])
            dum = wk.tile([16, 256], F32, tag="dum", name="dum")
            acc = wk.tile([16, 1], F32, tag="acc", name="acc")
            nc.scalar.activation(dum[:], hb_[:], mybir.ActivationFunctionType.Copy,
                                 accum_out=acc[:])
            mu = wk.tile([16, 1], F32, tag="mu", name="mu")
            nc.vector.tensor_scalar_mul(mu[:], acc[:], 1.0 / H)
            cen = wk.tile([16, 256], F32, tag="cen", name="cen")
            nc.vector.tensor_scalar_sub(cen[:], hb_[:], mu[:])
            acc2 = wk.tile([16, 1], F32, tag="acc2", name="acc2")
            nc.scalar.activation(dum[:], cen[:], mybir.ActivationFunctionType.Square,
                                 bias=zb[0:16, :], accum_out=acc2[:])
            sd = wk.tile([16, 1], F32, tag="sd", name="sd")
            nc.scalar.activation(sd[:], acc2[:], mybir.ActivationFunctionType.Sqrt,
                                 scale=1.0 / H, bias=eps16[:])
            rstd = wk.tile([16, 1], F32, tag="rstd", name="rstd")
            nc.vector.reciprocal(rstd[:], sd[:])
            nrm = wk.tile([16, 256], F32, tag="nrm", name="nrm")
            nc.vector.tensor_scalar_mul(nrm[:], cen[:], rstd[:])
            gam = wk.tile([16, 256], F32, tag="gam", name="gam")
            nc.sync.dma_start(gam[:], gam_d[:])
            bet = wk.tile([16, 256], F32, tag="bet", name="bet")
            nc.sync.dma_start(bet[:], bet_d[:])
            nc.vector.tensor_mul(nrm[:], nrm[:], gam[:])
            out = wk.tile([16, 256], F32, tag="out", name="out")
            nc.vector.tensor_add(out[:], nrm[:], bet[:])
            nc.sync.dma_start(y_d[:], out[:])

    if split_waits:
        _split_excess_waits(nc)
    return nc


def prep_inputs(x, W_ih, W_hh, b_ih, b_hh, ln_gamma, ln_beta, t_steps=T):
    """host-side shard + transpose + cast + rescale. Per-core input dicts.

    Rescaling for the all-sigmoid cell (tanh(v) = 2*sig(2v)-1, h stored
    as h/2):
      - g-gate rows of W_ih, W_hh, bias doubled (sig sees 2*xg)
      - Whh columns doubled (consumes h/2), layer-2 Wih columns doubled
    """
    bf = ml_dtypes.bfloat16
    Wih = np.asarray(W_ih, dtype=np.float64)[:, PERM, :].copy()
    Whh = np.asarray(W_hh, dtype=np.float64)[:, PERM, :].copy()
    bias = (np.asarray(b_ih, dtype=np.float64) + np.asarray(b_hh, dtype=np.float64))[:, PERM].copy()
    # g rows doubled (g block = rows 3H:4H after PERM)
    Wih[:, 3 * H:, :] *= 2.0
    Whh[:, 3 * H:, :] *= 2.0
    bias[:, 3 * H:] *= 2.0
    # h/2 compensation on consumer columns
    Whh *= 2.0
    Wih[1] *= 2.0

    wih = np.ascontiguousarray(np.transpose(Wih, (0, 2, 1))).reshape(NUM_LAYERS, 2, 128, G4)
    whh = np.ascontiguousarray(np.transpose(Whh, (0, 2, 1))).reshape(NUM_LAYERS, 2, 128, G4)
    # bias broadcast tile: bbrd[l, p, 128*ch + c] = bias[l, 128*ch + p]
    b3 = np.transpose(bias.reshape(NUM_LAYERS, NCH, 128), (0, 2, 1))  # [L,128,8]
    bbrd = np.ascontiguousarray(
        np.broadcast_to(b3[:, :, :, None], (NUM_LAYERS, 128, NCH, 128))
    ).reshape(NUM_LAYERS, 128, G4)
    ident = np.eye(128, dtype=np.float32)
    ins = []
    for cid in range(N_CORES):
        xs = x[cid * BL:(cid + 1) * BL, :t_steps, :]        # [16, t, 256]
        xtp = np.transpose(xs, (2, 1, 0)).reshape(F, t_steps * BL)  # [256, t*16]
        ins.append({
            "xt": np.ascontiguousarray(xtp.reshape(2, 128, t_steps * BL)).astype(bf),
            "wih": wih.astype(bf), "whh": whh.astype(bf),
            "bbrd": bbrd.astype(bf), "identb": ident.astype(bf), "ident": ident,
            "gam": np.broadcast_to(ln_gamma, (BL, H)).astype(np.float32).copy(),
            "bet": np.broadcast_to(ln_beta, (BL, H)).astype(np.float32).copy(),
        })
    return ins


_CACHED = {}


def kernel(x, W_ih, W_hh, b_ih, b_hh, ln_gamma, ln_beta):
    from concourse.bass_utils import run_bass_kernel_spmd
    x = np.asarray(x, dtype=np.float32)
    ins = prep_inputs(np.asarray(x), np.asarray(W_ih), np.asarray(W_hh),
                      np.asarray(b_ih), np.asarray(b_hh),
                      np.asarray(ln_gamma), np.asarray(ln_beta))
    if "nc" not in _CACHED:
        _CACHED["nc"] = build(T)
    res = run_bass_kernel_spmd(_CACHED["nc"], ins, core_ids=list(range(N_CORES)))
    return np.concatenate([res.results[c]["y"] for c in range(N_CORES)], axis=0)


# revision 25
# speedup vs baseline: 1.4827x; 1.0004x over previous
"""2-layer LSTM (B=128, T=1024, H=256) + last-step LayerNorm on 8 trn2 cores.

Data-parallel over batch (16 rows/core). Per core, everything is kept in a
transposed layout (hidden/gate dims on partitions, batch on the free axis):

  - gates.T for a block of 8 timesteps live in one PSUM tile [128, 1024]
    (col = 128*chunk + 16*step_in_block + b). Per block, the bias is first
    seeded with a single [8,128]-stationary x one-hot matmul (1 ldweights
    instead of 8), then the input projection x @ Wih.T is matmul-accumulated
    (Wih.T chunks stationary, x.T streaming). Each timestep's recurrent term
    then accumulates into its 16-column slice (Whh.T chunks stationary, h.T
    streaming), so no separate add is ever needed.
  - all four gates use ONE sigmoid instruction per step: tanh(x) is computed
    as 2*sigmoid(2x)-1 by pre-doubling the g-gate rows of the weights on the
    host, and h is stored as h/2 = (sig(2c)-0.5)*o with the missing factor 2
    folded into the next consumer's weight columns (Whh, layer-2 Wih); the
    final LayerNorm is scale-invariant so h/2 needs no correction there.
  - cell update per step: DVE ig2=(sg-.5)*i, Pool fc=f*c, DVE c=(2*ig2)+fc,
    ACT sc=sig(2c), DVE h=(sc-.5)*o written in bf16 directly where the next
    matmul streams it from.
  - layer 2 runs 8 steps behind layer 1 so its per-8-step x-projection
    (from layer 1's h history ring) is ready, and so PE/ACT/DVE work of
    the two layers overlaps.

Matmuls are bf16 (fp32 PSUM accumulate); c stays fp32. Final step: PE
transpose of h2 back to [16, 256], LayerNorm, DMA out.
"""
import sys

sys.path.insert(0, "/opt/trn_rl_repo")

import numpy as np
import ml_dtypes

import concourse.bass as bass
import concourse.mybir as mybir
import concourse.tile as tile
from concourse.alu_op_type import AluOpType

NUM_LAYERS = 2
H = 256
F = 256
B, T = 128, 1024
LN_EPS = 1e-5
N_CORES = 8
BL = B // N_CORES          # batch rows per core = 16
G4 = 4 * H                 # 1024 gate dims
NCH = G4 // 128            # 8 chunks of gate dims
SBLK = 8                   # timesteps per x-projection block
BF16 = mybir.dt.bfloat16
F32 = mybir.dt.float32

# gate order: torch (i,f,g,o) kept as-is — chunks (i0,i1,f0,f1,g0,g1,o0,o1).
# The main sigmoid covers i,f,g (chunks 0-5, on the critical path); the
# o-gate sigmoid (chunks 6-7) is only needed by the final h-multiply.
PERM = np.arange(4 * H)


def _split_excess_waits(nc):
    """walrus in this container rejects instructions with >2 sem waits
    (CoreV3 setupSyncWait). Hoist excess waits onto NoOps just before."""
    for fn in nc.m.functions:
        for blk in fn.blocks:
            insts = list(blk.instructions)
            out, n_new = [], 0
            for inst in insts:
                si = inst.sync_info
                waits = list(si.on_wait) if si is not None else []
                if len(waits) > 1:
                    head, rest = waits[:-1], waits[-1:]
                    # chain NoOps, one wait each (1-wait-per-inst walrus limit)
                    for wt in head:
                        nop = mybir.InstNoOp(
                            name=f"{inst.name}-ws{n_new}",
                            engine=inst.engine,
                            ins=[], outs=[],
                            sync_info=mybir.SyncInfo(on_wait=[wt], on_update=[]),
                        )
                        n_new += 1
                        out.append(nop)
                    inst.sync_info = mybir.SyncInfo(
                        on_wait=rest, on_update=list(si.on_update))
                out.append(inst)
            if n_new:
                try:
                    blk.instructions = out
                except Exception:
                    blk.set_instructions(out)


def build(t_steps=T, split_waits=True):
    nc = bass.Bass()
    TB = t_steps * BL
    xt_d = nc.dram_tensor("xt", [2, 128, TB], BF16, kind="ExternalInput")
    wih_d = nc.dram_tensor("wih", [NUM_LAYERS, 2, 128, G4], BF16, kind="ExternalInput")
    whh_d = nc.dram_tensor("whh", [NUM_LAYERS, 2, 128, G4], BF16, kind="ExternalInput")
    bbrd_d = nc.dram_tensor("bbrd", [NUM_LAYERS, 128, G4], BF16, kind="ExternalInput")
    identb_d = nc.dram_tensor("identb", [128, 128], BF16, kind="ExternalInput")
    ident_d = nc.dram_tensor("ident", [128, 128], F32, kind="ExternalInput")
    gam_d = nc.dram_tensor("gam", [BL, H], F32, kind="ExternalInput")
    bet_d = nc.dram_tensor("bet", [BL, H], F32, kind="ExternalInput")
    y_d = nc.dram_tensor("y", [BL, H], F32, kind="ExternalOutput")

    NB = t_steps // SBLK
    LAG = 12                   # layer-2 wave offset (staggers block xprojs)
    NSLOT = 2 * SBLK           # double-buffered layer-1 h history ring
    with tile.TileContext(nc) as tc:
        with (
            tc.tile_pool(name="wts", bufs=1) as wts,
            tc.tile_pool(name="state", bufs=1) as st,
            tc.tile_pool(name="work", bufs=4) as wk,
            tc.tile_pool(name="psum", bufs=2, space="PSUM") as ps,
        ):
            # resident tensors (partition dim first on every SBUF tile)
            xt = [wts.tile([128, TB], BF16, tag=f"xt{kw}", name=f"xt{kw}") for kw in (0, 1)]
            for kw in (0, 1):
                nc.sync.dma_start(xt[kw][:], xt_d[kw])
            wih = [[wts.tile([128, G4], BF16, tag=f"wih{l}{kw}", name=f"wih{l}{kw}") for kw in (0, 1)]
                   for l in range(NUM_LAYERS)]
            whh = [[wts.tile([128, G4], BF16, tag=f"whh{l}{kw}", name=f"whh{l}{kw}") for kw in (0, 1)]
                   for l in range(NUM_LAYERS)]
            bbrd = [wts.tile([128, G4], BF16, tag=f"bbrd{l}", name=f"bbrd{l}") for l in range(NUM_LAYERS)]
            identb = wts.tile([128, 128], BF16, tag="identb", name="identb")
            nc.sync.dma_start(identb[:], identb_d[:])
            for l in range(NUM_LAYERS):
                for kw in (0, 1):
                    nc.sync.dma_start(wih[l][kw][:], wih_d[l, kw])
                    nc.sync.dma_start(whh[l][kw][:], whh_d[l, kw])
                nc.sync.dma_start(bbrd[l][:], bbrd_d[l])
            ident = wts.tile([128, 128], F32, tag="ident", name="ident")
            nc.sync.dma_start(ident[:], ident_d[:])
            zb = wts.tile([128, 1], F32, tag="zb", name="zb")
            nc.vector.memset(zb[:], 0.0)
            # LN input is h/2, so var is scaled by 1/4 — scale eps to match
            eps16 = wts.tile([16, 1], F32, tag="eps16", name="eps16")
            nc.vector.memset(eps16[:], LN_EPS / 4)

            # persistent state
            hist = st.tile([128, NSLOT * 32], BF16, tag="hist", name="hist")  # layer-1 h ring
            h2 = st.tile([128, 32], BF16, tag="h2", name="h2")
            c1 = st.tile([128, 32], F32, tag="c1", name="c1")
            c2 = st.tile([128, 32], F32, tag="c2", name="c2")
            h2f = st.tile([128, 32], F32, tag="h2f", name="h2f")

            xp_cur = [None, None]    # psum tile being consumed, per layer
            xp_next = [None, None]   # psum tile being produced, per layer

            def xproj_rhs(l, k):
                if l == 0:
                    return [xt[kw][:, k * SBLK * BL:(k + 1) * SBLK * BL] for kw in (0, 1)]
                hv = hist[:].rearrange("p (s w) -> p s w", s=NSLOT)
                s0 = (k % 2) * SBLK
                return [hv[:, s0:s0 + SBLK, 0:BL], hv[:, s0:s0 + SBLK, BL:2 * BL]]

            def xproj_chunk(l, k, ch):
                """emit bias-seed (ch 0/4) + Wih matmuls for one gate chunk
                of block k into xp_next[l]."""
                if ch == 0:
                    xp_next[l] = ps.tile([128, SBLK * 128], F32, tag=f"xp{l}", name=f"xp{l}")
                xp = xp_next[l]
                if ch % 4 == 0:
                    hb = ch // 4
                    nc.tensor.matmul(
                        xp[:, hb * 512:(hb + 1) * 512], identb[:],
                        bbrd[l][:, hb * 512:(hb + 1) * 512],
                        start=True, stop=False, skip_group_check=True)
                rhs = xproj_rhs(l, k)
                o = xp[:, ch * 128:(ch + 1) * 128]
                for kw in (0, 1):
                    nc.tensor.matmul(
                        o, wih[l][kw][:, ch * 128:(ch + 1) * 128], rhs[kw],
                        start=False, stop=(kw == 1 and ch == NCH - 1),
                        skip_group_check=True)

            def rec_matmuls(l, t):
                """i,f,g chunks (0-5) first so the main sigmoid can launch
                while the o chunks (6-7) are still streaming; kw-major
                within each group so kw0 starts on the h half-0 write."""
                xp = xp_cur[l]
                s = t % SBLK
                if l == 0:
                    hsrc = hist[:, ((t - 1) % NSLOT) * 32:((t - 1) % NSLOT) * 32 + 32]
                else:
                    hsrc = h2[:]
                for grp in (range(0, 6), range(6, NCH)):
                    for kw in (0, 1):
                        for ch in grp:
                            o = xp[:, ch * 128 + 16 * s: ch * 128 + 16 * s + 16]
                            nc.tensor.matmul(
                                o, whh[l][kw][:, ch * 128:(ch + 1) * 128],
                                hsrc[:, 16 * kw:16 * kw + 16],
                                start=False, stop=(kw == 1), skip_group_check=True)

            def emit_sig(l, t):
                """main sigmoid: i,f,g chunks -> bf16 [128, 96]"""
                s = t % SBLK
                xpv = xp_cur[l][:].rearrange("p (c s w) -> p c s w", c=NCH, s=SBLK)
                sig = wk.tile([128, 96], F32, tag=f"sig{l}", name=f"sig{l}")
                nc.scalar.activation(sig[:].rearrange("p (c w) -> p c w", c=6),
                                     xpv[:, 0:6, s, :],
                                     mybir.ActivationFunctionType.Sigmoid,
                                     bias=zb[:])
                return sig

            def emit_sigo(l, t):
                """o-gate sigmoid: chunks 6-7 -> bf16 [128, 32] (off-chain)"""
                s = t % SBLK
                xpv = xp_cur[l][:].rearrange("p (c s w) -> p c s w", c=NCH, s=SBLK)
                sg = wk.tile([128, 32], BF16, tag=f"sigo{l}", name=f"sigo{l}")
                nc.scalar.activation(sg[:].rearrange("p (c w) -> p c w", c=2),
                                     xpv[:, 6:NCH, s, :],
                                     mybir.ActivationFunctionType.Sigmoid,
                                     bias=zb[:])
                return sg

            # prologue: layer-1 block 0 (layer-2 block 0 is emitted at
            # waves 8..11 once the hist ring has filled)
            for ch in range(NCH):
                xproj_chunk(0, 0, ch)

            for w in range(t_steps + LAG):
                act = []
                if LAG <= w:
                    act.append((1, w - LAG))
                if w < t_steps:
                    act.append((0, w))

                # consume-switch psum blocks at each layer's block start
                for (l, t) in act:
                    if t % SBLK == 0:
                        xp_cur[l] = xp_next[l]

                # stage B: recurrent matmuls (l2 first: its inputs are older)
                for (l, t) in act:
                    if t > 0:
                        rec_matmuls(l, t)

                # stage A: xproj slices for upcoming blocks (PE, after recs).
                # Emitted in the LATE half of each block's 8 waves so the
                # psum-buffer WAR (bufs=2 rotation) has drained.
                if w % SBLK >= 4 and w < t_steps:
                    k1 = w // SBLK + 1
                    if k1 < NB:
                        for ch in (2 * (w % SBLK - 4), 2 * (w % SBLK - 4) + 1):
                            xproj_chunk(0, k1, ch)
                if w >= SBLK and w % SBLK <= 3:
                    k2 = (w - SBLK) // SBLK
                    if k2 < NB:
                        for ch in (2 * (w % SBLK), 2 * (w % SBLK) + 1):
                            xproj_chunk(1, k2, ch)

                # stage C: main sigmoids (i,f,g), then o-gate sigmoids
                sigs = {l: emit_sig(l, t) for (l, t) in act}
                sigos = {l: emit_sigo(l, t) for (l, t) in act}

                # stages D/E: all-DVE cell chain per layer (back-to-back on
                # one engine avoids two cross-engine sem hops via Pool):
                #   ig2 = (sg-0.5)*i ; fc = f*c ; c = 2*ig2 + fc
                for (l, t) in act:
                    sig = sigs[l]
                    c_t = c1 if l == 0 else c2
                    ig2 = wk.tile([128, 32], F32, tag=f"ig2{l}", name=f"ig2{l}")
                    nc.vector.scalar_tensor_tensor(
                        ig2[:], sig[:, 64:96], 0.5, sig[:, 0:32],
                        AluOpType.subtract, AluOpType.mult)
                    if t > 0:
                        fc = wk.tile([128, 32], F32, tag=f"fc{l}", name=f"fc{l}")
                        nc.vector.tensor_mul(fc[:], sig[:, 32:64], c_t[:])
                        nc.vector.scalar_tensor_tensor(
                            c_t[:], ig2[:], 2.0, fc[:],
                            AluOpType.mult, AluOpType.add)
                    else:
                        nc.vector.tensor_scalar_mul(c_t[:], ig2[:], 2.0)

                # stage F: tc = tanh(c) (ACT)
                tcs = {}
                for (l, t) in act:
                    c_t = c1 if l == 0 else c2
                    sc = wk.tile([128, 32], F32, tag=f"sc{l}", name=f"sc{l}")
                    nc.scalar.activation(sc[:], c_t[:],
                                         mybir.ActivationFunctionType.Tanh,
                                         bias=zb[:])
                    tcs[l] = sc

                # stage G: h/2 = (tc*0.5)*o (DVE), bf16 into next consumer.
                # Written in two 16-col halves so the next step's kw0 rec
                # matmuls can launch while the kw1 half is still in flight.
                for (l, t) in act:
                    if l == 0:
                        hdst = hist[:, (t % NSLOT) * 32:(t % NSLOT) * 32 + 32]
                    else:
                        hdst = h2[:]
                    # half-0 on DVE (fast path: next kw0 recs wait on it);
                    # half-1 on Pool — its extra latency hides under the
                    # 8 kw0 matmuls that run first.
                    nc.vector.scalar_tensor_tensor(
                        hdst[:, 0:16], tcs[l][:, 0:16], 0.5,
                        sigos[l][:, 0:16],
                        AluOpType.mult, AluOpType.mult)
                    nc.gpsimd.scalar_tensor_tensor(
                        hdst[:, 16:32], tcs[l][:, 16:32], 0.5,
                        sigos[l][:, 16:32],
                        AluOpType.mult, AluOpType.mult)
                    if l == 1 and t == t_steps - 1:
                        nc.vector.scalar_tensor_tensor(
                            h2f[:], tcs[l][:], 0.5, sigos[l][:],
                            AluOpType.mult, AluOpType.mult)

            # ---- LayerNorm over H on h2f (h2.T layout, h/2 scale) -> y [16, 256]
            pt = ps.tile([16, 256], F32, tag="xp0", name="xp0")
            nc.tensor.transpose(pt[:, 0:128], h2f[:, 0:16], ident[:])
            nc.tensor.transpose(pt[:, 128:256], h2f[:, 16:32], ident[:])
            hb_ = wk.tile([16, 256], F32, tag="hb", name="hb")
            nc.vector.tensor_copy(hb_[:], pt[:])
            dum = wk.tile([16, 256], F32, tag="dum", name="dum")
            acc = wk.tile([16, 1], F32, tag="acc", name="acc")
            nc.scalar.activation(dum[:], hb_[:], mybir.ActivationFunctionType.Copy,
                                 accum_out=acc[:])
            mu = wk.tile([16, 1], F32, tag="mu", name="mu")
            nc.vector.tensor_scalar_mul(mu[:], acc[:], 1.0 / H)
            cen = wk.tile([16, 256], F32, tag="cen", name="cen")
            nc.vector.tensor_scalar_sub(cen[:], hb_[:], mu[:])
            acc2 = wk.tile([16, 1], F32, tag="acc2", name="acc2")
            nc.scalar.activation(dum[:], cen[:], mybir.ActivationFunctionType.Square,
                                 bias=zb[0:16, :], accum_out=acc2[:])
            sd = wk.tile([16, 1], F32, tag="sd", name="sd")
            nc.scalar.activation(sd[:], acc2[:], mybir.ActivationFunctionType.Sqrt,
                                 scale=1.0 / H, bias=eps16[:])
            rstd = wk.tile([16, 1], F32, tag="rstd", name="rstd")
            nc.vector.reciprocal(rstd[:], sd[:])
            nrm = wk.tile([16, 256], F32, tag="nrm", name="nrm")
            nc.vector.tensor_scalar_mul(nrm[:], cen[:], rstd[:])
            gam = wk.tile([16, 256], F32, tag="gam", name="gam")
            nc.sync.dma_start(gam[:], gam_d[:])
            bet = wk.tile([16, 256], F32, tag="bet", name="bet")
            nc.sync.dma_start(bet[:], bet_d[:])
            nc.vector.tensor_mul(nrm[:], nrm[:], gam[:])
            out = wk.tile([16, 256], F32, tag="out", name="out")
            nc.vector.tensor_add(out[:], nrm[:], bet[:])
            nc.sync.dma_start(y_d[:], out[:])

    if split_waits:
        _split_excess_waits(nc)
    return nc


def prep_inputs(x, W_ih, W_hh, b_ih, b_hh, ln_gamma, ln_beta, t_steps=T):
    """host-side shard + transpose + cast + rescale. Per-core input dicts.

    Rescaling for the all-sigmoid cell (tanh(v) = 2*sig(2v)-1, h stored
    as h/2):
      - g-gate rows of W_ih, W_hh, bias doubled (sig sees 2*xg)
      - Whh columns doubled (consumes h/2), layer-2 Wih columns doubled
    """
    bf = ml_dtypes.bfloat16
    Wih = np.asarray(W_ih, dtype=np.float64)[:, PERM, :].copy()
    Whh = np.asarray(W_hh, dtype=np.float64)[:, PERM, :].copy()
    bias = (np.asarray(b_ih, dtype=np.float64) + np.asarray(b_hh, dtype=np.float64))[:, PERM].copy()
    # g rows doubled (g block = rows 2H:3H in torch order)
    Wih[:, 2 * H:3 * H, :] *= 2.0
    Whh[:, 2 * H:3 * H, :] *= 2.0
    bias[:, 2 * H:3 * H] *= 2.0
    # h/2 compensation on consumer columns
    Whh *= 2.0
    Wih[1] *= 2.0

    wih = np.ascontiguousarray(np.transpose(Wih, (0, 2, 1))).reshape(NUM_LAYERS, 2, 128, G4)
    whh = np.ascontiguousarray(np.transpose(Whh, (0, 2, 1))).reshape(NUM_LAYERS, 2, 128, G4)
    # bias broadcast tile: bbrd[l, p, 128*ch + c] = bias[l, 128*ch + p]
    b3 = np.transpose(bias.reshape(NUM_LAYERS, NCH, 128), (0, 2, 1))  # [L,128,8]
    bbrd = np.ascontiguousarray(
        np.broadcast_to(b3[:, :, :, None], (NUM_LAYERS, 128, NCH, 128))
    ).reshape(NUM_LAYERS, 128, G4)
    ident = np.eye(128, dtype=np.float32)
    ins = []
    for cid in range(N_CORES):
        xs = x[cid * BL:(cid + 1) * BL, :t_steps, :]        # [16, t, 256]
        xtp = np.transpose(xs, (2, 1, 0)).reshape(F, t_steps * BL)  # [256, t*16]
        ins.append({
            "xt": np.ascontiguousarray(xtp.reshape(2, 128, t_steps * BL)).astype(bf),
            "wih": wih.astype(bf), "whh": whh.astype(bf),
            "bbrd": bbrd.astype(bf), "identb": ident.astype(bf), "ident": ident,
            "gam": np.broadcast_to(ln_gamma, (BL, H)).astype(np.float32).copy(),
            "bet": np.broadcast_to(ln_beta, (BL, H)).astype(np.float32).copy(),
        })
    return ins


_CACHED = {}


def kernel(x, W_ih, W_hh, b_ih, b_hh, ln_gamma, ln_beta):
    from concourse.bass_utils import run_bass_kernel_spmd
    x = np.asarray(x, dtype=np.float32)
    ins = prep_inputs(np.asarray(x), np.asarray(W_ih), np.asarray(W_hh),
                      np.asarray(b_ih), np.asarray(b_hh),
                      np.asarray(ln_gamma), np.asarray(ln_beta))
    if "nc" not in _CACHED:
        _CACHED["nc"] = build(T)
    res = run_bass_kernel_spmd(_CACHED["nc"], ins, core_ids=list(range(N_CORES)))
    return np.concatenate([res.results[c]["y"] for c in range(N_CORES)], axis=0)


# revision 27
# speedup vs baseline: 1.4844x; 1.0011x over previous
"""2-layer LSTM (B=128, T=1024, H=256) + last-step LayerNorm on 8 trn2 cores.

Data-parallel over batch (16 rows/core). Per core, everything is kept in a
transposed layout (hidden/gate dims on partitions, batch on the free axis):

  - gates.T for a block of 8 timesteps live in one PSUM tile [128, 1024]
    (col = 128*chunk + 16*step_in_block + b). Per block, the bias is first
    seeded with a single [8,128]-stationary x one-hot matmul (1 ldweights
    instead of 8), then the input projection x @ Wih.T is matmul-accumulated
    (Wih.T chunks stationary, x.T streaming). Each timestep's recurrent term
    then accumulates into its 16-column slice (Whh.T chunks stationary, h.T
    streaming), so no separate add is ever needed.
  - all four gates use ONE sigmoid instruction per step: tanh(x) is computed
    as 2*sigmoid(2x)-1 by pre-doubling the g-gate rows of the weights on the
    host, and h is stored as h/2 = (sig(2c)-0.5)*o with the missing factor 2
    folded into the next consumer's weight columns (Whh, layer-2 Wih); the
    final LayerNorm is scale-invariant so h/2 needs no correction there.
  - cell update per step: DVE ig2=(sg-.5)*i, Pool fc=f*c, DVE c=(2*ig2)+fc,
    ACT sc=sig(2c), DVE h=(sc-.5)*o written in bf16 directly where the next
    matmul streams it from.
  - layer 2 runs 8 steps behind layer 1 so its per-8-step x-projection
    (from layer 1's h history ring) is ready, and so PE/ACT/DVE work of
    the two layers overlaps.

Matmuls are bf16 (fp32 PSUM accumulate); c stays fp32. Final step: PE
transpose of h2 back to [16, 256], LayerNorm, DMA out.
"""
import sys

sys.path.insert(0, "/opt/trn_rl_repo")

import numpy as np
import ml_dtypes

import concourse.bass as bass
import concourse.mybir as mybir
import concourse.tile as tile
from concourse.alu_op_type import AluOpType

NUM_LAYERS = 2
H = 256
F = 256
B, T = 128, 1024
LN_EPS = 1e-5
N_CORES = 8
BL = B // N_CORES          # batch rows per core = 16
G4 = 4 * H                 # 1024 gate dims
NCH = G4 // 128            # 8 chunks of gate dims
SBLK = 8                   # timesteps per x-projection block
BF16 = mybir.dt.bfloat16
F32 = mybir.dt.float32

# gate order: torch (i,f,g,o) kept as-is — chunks (i0,i1,f0,f1,g0,g1,o0,o1).
# The main sigmoid covers i,f,g (chunks 0-5, on the critical path); the
# o-gate sigmoid (chunks 6-7) is only needed by the final h-multiply.
PERM = np.arange(4 * H)


def _split_excess_waits(nc):
    """walrus in this container rejects instructions with >2 sem waits
    (CoreV3 setupSyncWait). Hoist excess waits onto NoOps just before."""
    for fn in nc.m.functions:
        for blk in fn.blocks:
            insts = list(blk.instructions)
            out, n_new = [], 0
            for inst in insts:
                si = inst.sync_info
                waits = list(si.on_wait) if si is not None else []
                if len(waits) > 1:
                    head, rest = waits[:-1], waits[-1:]
                    # chain NoOps, one wait each (1-wait-per-inst walrus limit)
                    for wt in head:
                        nop = mybir.InstNoOp(
                            name=f"{inst.name}-ws{n_new}",
                            engine=inst.engine,
                            ins=[], outs=[],
                            sync_info=mybir.SyncInfo(on_wait=[wt], on_update=[]),
                        )
                        n_new += 1
                        out.append(nop)
                    inst.sync_info = mybir.SyncInfo(
                        on_wait=rest, on_update=list(si.on_update))
                out.append(inst)
            if n_new:
                try:
                    blk.instructions = out
                except Exception:
                    blk.set_instructions(out)


def build(t_steps=T, split_waits=True):
    nc = bass.Bass()
    TB = t_steps * BL
    xt_d = nc.dram_tensor("xt", [2, 128, TB], BF16, kind="ExternalInput")
    wih_d = nc.dram_tensor("wih", [NUM_LAYERS, 2, 128, G4], BF16, kind="ExternalInput")
    whh_d = nc.dram_tensor("whh", [NUM_LAYERS, 2, 128, G4], BF16, kind="ExternalInput")
    bbrd_d = nc.dram_tensor("bbrd", [NUM_LAYERS, 128, G4], BF16, kind="ExternalInput")
    identb_d = nc.dram_tensor("identb", [128, 128], BF16, kind="ExternalInput")
    ident_d = nc.dram_tensor("ident", [128, 128], F32, kind="ExternalInput")
    gam_d = nc.dram_tensor("gam", [BL, H], F32, kind="ExternalInput")
    bet_d = nc.dram_tensor("bet", [BL, H], F32, kind="ExternalInput")
    y_d = nc.dram_tensor("y", [BL, H], F32, kind="ExternalOutput")

    NB = t_steps // SBLK
    LAG = 12                   # layer-2 wave offset (staggers block xprojs)
    NSLOT = 2 * SBLK           # double-buffered layer-1 h history ring
    with tile.TileContext(nc) as tc:
        with (
            tc.tile_pool(name="wts", bufs=1) as wts,
            tc.tile_pool(name="state", bufs=1) as st,
            tc.tile_pool(name="work", bufs=4) as wk,
            tc.tile_pool(name="psum", bufs=2, space="PSUM") as ps,
        ):
            # resident tensors (partition dim first on every SBUF tile)
            xt = [wts.tile([128, TB], BF16, tag=f"xt{kw}", name=f"xt{kw}") for kw in (0, 1)]
            for kw in (0, 1):
                nc.sync.dma_start(xt[kw][:], xt_d[kw])
            wih = [[wts.tile([128, G4], BF16, tag=f"wih{l}{kw}", name=f"wih{l}{kw}") for kw in (0, 1)]
                   for l in range(NUM_LAYERS)]
            whh = [[wts.tile([128, G4], BF16, tag=f"whh{l}{kw}", name=f"whh{l}{kw}") for kw in (0, 1)]
                   for l in range(NUM_LAYERS)]
            bbrd = [wts.tile([128, G4], BF16, tag=f"bbrd{l}", name=f"bbrd{l}") for l in range(NUM_LAYERS)]
            identb = wts.tile([128, 128], BF16, tag="identb", name="identb")
            nc.sync.dma_start(identb[:], identb_d[:])
            for l in range(NUM_LAYERS):
                for kw in (0, 1):
                    nc.sync.dma_start(wih[l][kw][:], wih_d[l, kw])
                    nc.sync.dma_start(whh[l][kw][:], whh_d[l, kw])
                nc.sync.dma_start(bbrd[l][:], bbrd_d[l])
            ident = wts.tile([128, 128], F32, tag="ident", name="ident")
            nc.sync.dma_start(ident[:], ident_d[:])
            zb = wts.tile([128, 1], F32, tag="zb", name="zb")
            nc.vector.memset(zb[:], 0.0)
            # LN input is h/2, so var is scaled by 1/4 — scale eps to match
            eps16 = wts.tile([16, 1], F32, tag="eps16", name="eps16")
            nc.vector.memset(eps16[:], LN_EPS / 4)

            # persistent state
            hist = st.tile([128, NSLOT * 32], BF16, tag="hist", name="hist")  # layer-1 h ring
            h2 = st.tile([128, 32], BF16, tag="h2", name="h2")
            c1 = st.tile([128, 32], F32, tag="c1", name="c1")
            c2 = st.tile([128, 32], F32, tag="c2", name="c2")
            h2f = st.tile([128, 32], F32, tag="h2f", name="h2f")

            xp_cur = [None, None]    # psum tile being consumed, per layer
            xp_next = [None, None]   # psum tile being produced, per layer

            def xproj_rhs(l, k):
                if l == 0:
                    return [xt[kw][:, k * SBLK * BL:(k + 1) * SBLK * BL] for kw in (0, 1)]
                hv = hist[:].rearrange("p (s w) -> p s w", s=NSLOT)
                s0 = (k % 2) * SBLK
                return [hv[:, s0:s0 + SBLK, 0:BL], hv[:, s0:s0 + SBLK, BL:2 * BL]]

            def xproj_chunk(l, k, ch):
                """emit bias-seed (ch 0/4) + Wih matmuls for one gate chunk
                of block k into xp_next[l]."""
                if ch == 0:
                    xp_next[l] = ps.tile([128, SBLK * 128], F32, tag=f"xp{l}", name=f"xp{l}")
                xp = xp_next[l]
                if ch % 4 == 0:
                    hb = ch // 4
                    nc.tensor.matmul(
                        xp[:, hb * 512:(hb + 1) * 512], identb[:],
                        bbrd[l][:, hb * 512:(hb + 1) * 512],
                        start=True, stop=False, skip_group_check=True)
                rhs = xproj_rhs(l, k)
                o = xp[:, ch * 128:(ch + 1) * 128]
                for kw in (0, 1):
                    nc.tensor.matmul(
                        o, wih[l][kw][:, ch * 128:(ch + 1) * 128], rhs[kw],
                        start=False, stop=(kw == 1 and ch == NCH - 1),
                        skip_group_check=True)

            def rec_matmuls(l, t):
                """i,f,g chunks (0-5) first so the main sigmoid can launch
                while the o chunks (6-7) are still streaming; kw-major
                within each group so kw0 starts on the h half-0 write."""
                xp = xp_cur[l]
                s = t % SBLK
                if l == 0:
                    hsrc = hist[:, ((t - 1) % NSLOT) * 32:((t - 1) % NSLOT) * 32 + 32]
                else:
                    hsrc = h2[:]
                for grp in (range(0, 6), range(6, NCH)):
                    for kw in (0, 1):
                        for ch in grp:
                            o = xp[:, ch * 128 + 16 * s: ch * 128 + 16 * s + 16]
                            nc.tensor.matmul(
                                o, whh[l][kw][:, ch * 128:(ch + 1) * 128],
                                hsrc[:, 16 * kw:16 * kw + 16],
                                start=False, stop=(kw == 1), skip_group_check=True)

            def emit_sig(l, t):
                """main sigmoid: i,f,g chunks -> bf16 [128, 96]"""
                s = t % SBLK
                xpv = xp_cur[l][:].rearrange("p (c s w) -> p c s w", c=NCH, s=SBLK)
                sig = wk.tile([128, 96], F32, tag=f"sig{l}", name=f"sig{l}")
                nc.scalar.activation(sig[:].rearrange("p (c w) -> p c w", c=6),
                                     xpv[:, 0:6, s, :],
                                     mybir.ActivationFunctionType.Sigmoid,
                                     bias=zb[:])
                return sig

            def emit_sigo(l, t):
                """o-gate sigmoid: chunks 6-7 -> bf16 [128, 32] (off-chain)"""
                s = t % SBLK
                xpv = xp_cur[l][:].rearrange("p (c s w) -> p c s w", c=NCH, s=SBLK)
                sg = wk.tile([128, 32], BF16, tag=f"sigo{l}", name=f"sigo{l}")
                nc.scalar.activation(sg[:].rearrange("p (c w) -> p c w", c=2),
                                     xpv[:, 6:NCH, s, :],
                                     mybir.ActivationFunctionType.Sigmoid,
                                     bias=zb[:])
                return sg

            # prologue: layer-1 block 0 (layer-2 block 0 is emitted at
            # waves 8..11 once the hist ring has filled)
            for ch in range(NCH):
                xproj_chunk(0, 0, ch)

            for w in range(t_steps + LAG):
                act = []
                if LAG <= w:
                    act.append((1, w - LAG))
                if w < t_steps:
                    act.append((0, w))

                # consume-switch psum blocks at each layer's block start
                for (l, t) in act:
                    if t % SBLK == 0:
                        xp_cur[l] = xp_next[l]

                # stage B: recurrent matmuls (l2 first: its inputs are older)
                for (l, t) in act:
                    if t > 0:
                        rec_matmuls(l, t)

                # stage A: xproj slices for upcoming blocks (PE, after recs).
                # Emitted in the LATE half of each block's 8 waves so the
                # psum-buffer WAR (bufs=2 rotation) has drained.
                if w % SBLK >= 4 and w < t_steps:
                    k1 = w // SBLK + 1
                    if k1 < NB:
                        for ch in (2 * (w % SBLK - 4), 2 * (w % SBLK - 4) + 1):
                            xproj_chunk(0, k1, ch)
                if w >= SBLK and w % SBLK <= 3:
                    k2 = (w - SBLK) // SBLK
                    if k2 < NB:
                        for ch in (2 * (w % SBLK), 2 * (w % SBLK) + 1):
                            xproj_chunk(1, k2, ch)

                # stage C: main sigmoids (i,f,g), then o-gate sigmoids
                sigs = {l: emit_sig(l, t) for (l, t) in act}
                sigos = {l: emit_sigo(l, t) for (l, t) in act}

                # stages D/E: all-DVE cell chain per layer (back-to-back on
                # one engine avoids two cross-engine sem hops via Pool):
                #   ig2 = (sg-0.5)*i ; fc = f*c ; c = 2*ig2 + fc
                for (l, t) in act:
                    sig = sigs[l]
                    c_t = c1 if l == 0 else c2
                    ig2 = wk.tile([128, 32], F32, tag=f"ig2{l}", name=f"ig2{l}")
                    nc.vector.scalar_tensor_tensor(
                        ig2[:], sig[:, 64:96], 0.5, sig[:, 0:32],
                        AluOpType.subtract, AluOpType.mult)
                    if t > 0:
                        fc = wk.tile([128, 32], F32, tag=f"fc{l}", name=f"fc{l}")
                        nc.vector.tensor_mul(fc[:], sig[:, 32:64], c_t[:])
                        nc.vector.scalar_tensor_tensor(
                            c_t[:], ig2[:], 2.0, fc[:],
                            AluOpType.mult, AluOpType.add)
                    else:
                        nc.vector.tensor_scalar_mul(c_t[:], ig2[:], 2.0)

                # stage F: tc = tanh(c) (ACT, bf16 out — h is bf16 anyway)
                tcs = {}
                for (l, t) in act:
                    c_t = c1 if l == 0 else c2
                    sc = wk.tile([128, 32], BF16, tag=f"sc{l}", name=f"sc{l}")
                    nc.scalar.activation(sc[:], c_t[:],
                                         mybir.ActivationFunctionType.Tanh,
                                         bias=zb[:])
                    tcs[l] = sc

                # stage G: h/2 = (tc*0.5)*o (DVE), bf16 into next consumer.
                # Written in two 16-col halves so the next step's kw0 rec
                # matmuls can launch while the kw1 half is still in flight.
                for (l, t) in act:
                    if l == 0:
                        hdst = hist[:, (t % NSLOT) * 32:(t % NSLOT) * 32 + 32]
                    else:
                        hdst = h2[:]
                    for kw in (0, 1):
                        nc.vector.scalar_tensor_tensor(
                            hdst[:, 16 * kw:16 * kw + 16],
                            tcs[l][:, 16 * kw:16 * kw + 16], 0.5,
                            sigos[l][:, 16 * kw:16 * kw + 16],
                            AluOpType.mult, AluOpType.mult)
                    if l == 1 and t == t_steps - 1:
                        nc.vector.scalar_tensor_tensor(
                            h2f[:], tcs[l][:], 0.5, sigos[l][:],
                            AluOpType.mult, AluOpType.mult)

            # ---- LayerNorm over H on h2f (h2.T layout, h/2 scale) -> y [16, 256]
            pt = ps.tile([16, 256], F32, tag="xp0", name="xp0")
            nc.tensor.transpose(pt[:, 0:128], h2f[:, 0:16], ident[:])
            nc.tensor.transpose(pt[:, 128:256], h2f[:, 16:32], ident[:])
            hb_ = wk.tile([16, 256], F32, tag="hb", name="hb")
            nc.vector.tensor_copy(hb_[:], pt[:])
            dum = wk.tile([16, 256], F32, tag="dum", name="dum")
            acc = wk.tile([16, 1], F32, tag="acc", name="acc")
            nc.scalar.activation(dum[:], hb_[:], mybir.ActivationFunctionType.Copy,
                                 accum_out=acc[:])
            mu = wk.tile([16, 1], F32, tag="mu", name="mu")
            nc.vector.tensor_scalar_mul(mu[:], acc[:], 1.0 / H)
            cen = wk.tile([16, 256], F32, tag="cen", name="cen")
            nc.vector.tensor_scalar_sub(cen[:], hb_[:], mu[:])
            acc2 = wk.tile([16, 1], F32, tag="acc2", name="acc2")
            nc.scalar.activation(dum[:], cen[:], mybir.ActivationFunctionType.Square,
                                 bias=zb[0:16, :], accum_out=acc2[:])
            sd = wk.tile([16, 1], F32, tag="sd", name="sd")
            nc.scalar.activation(sd[:], acc2[:], mybir.ActivationFunctionType.Sqrt,
                                 scale=1.0 / H, bias=eps16[:])
            rstd = wk.tile([16, 1], F32, tag="rstd", name="rstd")
            nc.vector.reciprocal(rstd[:], sd[:])
            nrm = wk.tile([16, 256], F32, tag="nrm", name="nrm")
            nc.vector.tensor_scalar_mul(nrm[:], cen[:], rstd[:])
            gam = wk.tile([16, 256], F32, tag="gam", name="gam")
            nc.sync.dma_start(gam[:], gam_d[:])
            bet = wk.tile([16, 256], F32, tag="bet", name="bet")
            nc.sync.dma_start(bet[:], bet_d[:])
            nc.vector.tensor_mul(nrm[:], nrm[:], gam[:])
            out = wk.tile([16, 256], F32, tag="out", name="out")
            nc.vector.tensor_add(out[:], nrm[:], bet[:])
            nc.sync.dma_start(y_d[:], out[:])

    if split_waits:
        _split_excess_waits(nc)
    return nc


def prep_inputs(x, W_ih, W_hh, b_ih, b_hh, ln_gamma, ln_beta, t_steps=T):
    """host-side shard + transpose + cast + rescale. Per-core input dicts.

    Rescaling for the all-sigmoid cell (tanh(v) = 2*sig(2v)-1, h stored
    as h/2):
      - g-gate rows of W_ih, W_hh, bias doubled (sig sees 2*xg)
      - Whh columns doubled (consumes h/2), layer-2 Wih columns doubled
    """
    bf = ml_dtypes.bfloat16
    Wih = np.asarray(W_ih, dtype=np.float64)[:, PERM, :].copy()
    Whh = np.asarray(W_hh, dtype=np.float64)[:, PERM, :].copy()
    bias = (np.asarray(b_ih, dtype=np.float64) + np.asarray(b_hh, dtype=np.float64))[:, PERM].copy()
    # g rows doubled (g block = rows 2H:3H in torch order)
    Wih[:, 2 * H:3 * H, :] *= 2.0
    Whh[:, 2 * H:3 * H, :] *= 2.0
    bias[:, 2 * H:3 * H] *= 2.0
    # h/2 compensation on consumer columns
    Whh *= 2.0
    Wih[1] *= 2.0

    wih = np.ascontiguousarray(np.transpose(Wih, (0, 2, 1))).reshape(NUM_LAYERS, 2, 128, G4)
    whh = np.ascontiguousarray(np.transpose(Whh, (0, 2, 1))).reshape(NUM_LAYERS, 2, 128, G4)
    # bias broadcast tile: bbrd[l, p, 128*ch + c] = bias[l, 128*ch + p]
    b3 = np.transpose(bias.reshape(NUM_LAYERS, NCH, 128), (0, 2, 1))  # [L,128,8]
    bbrd = np.ascontiguousarray(
        np.broadcast_to(b3[:, :, :, None], (NUM_LAYERS, 128, NCH, 128))
    ).reshape(NUM_LAYERS, 128, G4)
    ident = np.eye(128, dtype=np.float32)
    ins = []
    for cid in range(N_CORES):
        xs = x[cid * BL:(cid + 1) * BL, :t_steps, :]        # [16, t, 256]
        xtp = np.transpose(xs, (2, 1, 0)).reshape(F, t_steps * BL)  # [256, t*16]
        ins.append({
            "xt": np.ascontiguousarray(xtp.reshape(2, 128, t_steps * BL)).astype(bf),
            "wih": wih.astype(bf), "whh": whh.astype(bf),
            "bbrd": bbrd.astype(bf), "identb": ident.astype(bf), "ident": ident,
            "gam": np.broadcast_to(ln_gamma, (BL, H)).astype(np.float32).copy(),
            "bet": np.broadcast_to(ln_beta, (BL, H)).astype(np.float32).copy(),
        })
    return ins


_CACHED = {}


def kernel(x, W_ih, W_hh, b_ih, b_hh, ln_gamma, ln_beta):
    from concourse.bass_utils import run_bass_kernel_spmd
    x = np.asarray(x, dtype=np.float32)
    ins = prep_inputs(np.asarray(x), np.asarray(W_ih), np.asarray(W_hh),
                      np.asarray(b_ih), np.asarray(b_hh),
                      np.asarray(ln_gamma), np.asarray(ln_beta))
    if "nc" not in _CACHED:
        _CACHED["nc"] = build(T)
    res = run_bass_kernel_spmd(_CACHED["nc"], ins, core_ids=list(range(N_CORES)))
    return np.concatenate([res.results[c]["y"] for c in range(N_CORES)], axis=0)
